# revision 23
# baseline (speedup 1.0000x reference)
"""AttentionConv1d Trainium2 kernel — 8-core batch-parallel SPMD.

Reference semantics (B=8, C=512, T=4096, O=512, K=3):
    out[b,o,t] = sum_{c,k} feature[b,c,t+k-1] * sim[b, (3c+k)//512, t] * weight[o,c,k]
where sim[b,0/1/2,t] are cosine similarities of embedding column t with its
left neighbor / itself / right neighbor (zero-padded at the edges), so
sim[:,1,:] == 1 for any column with norm >= eps.

Decomposition per batch element (one NeuronCore each):
    conv_j[o,t] = sum over the (c,k) pairs with (3c+k)//512 == j
    out = sim_l * conv_0 + conv_1 + sim_r * conv_2
computed transposed (out_T[t,o]) so sim_l/sim_r become per-partition scalars.

Exact 12-chunk packing: each j-group's 512 (c,k) pairs split into 3 dense
128-channel chunks (k = 0,1,2 column offsets of one resident F tile) plus one
128-row boundary chunk whose rows are host-side k-shifted copies — no masked
zero rows, 12 matmuls of 128 contraction rows per 128-t out tile (the 1536-row
minimum). DMA priority: E (bf16) first so the sim reduce starts immediately,
then W, then F in column blocks so conv tile q never waits. The conv runs in
bf16 (host-cast), PSUM accumulation in fp32.
"""
from contextlib import ExitStack

import ml_dtypes
import numpy as np

import concourse.bass as bass
import concourse.tile as tile
from concourse import mybir
from concourse.bass_utils import run_bass_kernel_spmd

F32 = mybir.dt.float32
BF16 = mybir.dt.bfloat16

B, C, T, O, K = 8, 512, 4096, 512, 3
CP = C // 128  # 4 c-tiles
TQ = T // 128  # 32 t-tiles
NKB = T // 1024  # 4 reduce kilo-blocks

# j-group (3c+k)//512 channel structure: dense 128-channel blocks + 128
# boundary pairs per group (exactly 512 pairs per group, 1536 total).
BIGS = [0, 171, 384]
D_PAIRS = [
    [(c, k) for c in range(128, 170) for k in range(3)] + [(170, 0), (170, 1)],
    [(170, 2)] + [(c, k) for c in range(299, 341) for k in range(3)] + [(341, 0)],
    [(341, 1), (341, 2)] + [(c, k) for c in range(342, 384) for k in range(3)],
]
assert all(len(p) == 128 for p in D_PAIRS)

# F column blocks (start, width) in f_pad coords; blocks overlap 2 cols so any
# 130-col conv window lies inside one block. First block is small so the conv
# can start as soon as ~0.4MB of F has landed.
F_BLOCKS = [(0, 258), (256, 770), (1024, 1026), (2048, 1026), (3072, 1026)]
F_BLK_Q = [(0, 2), (2, 8), (8, 16), (16, 24), (24, 32)]  # q-tile range per block
_F_STARTS = [0]
for _a, _w in F_BLOCKS:
    _F_STARTS.append(_F_STARTS[-1] + 6 * _w)  # fmega col start per block
F_COLS = _F_STARTS[-1]  # 24636

# wmega chunk order: j0 (4), j2 (4), j1 (4)
W_COL = {}
for _i, _ci in enumerate([0, 1, 2, 3, 8, 9, 10, 11, 4, 5, 6, 7]):
    W_COL[_ci] = 512 * _i


def host_prep(feature, embedding, weight):
    """Build per-core input maps: packed F/E shards + exact packed weights.

    Everything is packed into 128-partition-row mega layouts so each tensor
    needs only a handful of dma_start instructions (each costs ~0.6us of sync
    engine issue time on HW):
      e_packed [128, 16400]: 4 kilo-blocks x 4 c-tiles x 1025 cols (bf16)
      fmega    [128, 24624]: 4 col-blocks x 6 chunk-tiles x 1026 cols, blocks
               overlap 2 cols so any 130-col conv window stays inside a block
      wmega    [128, 6144]: 12 chunks x 512 out-channels
    """
    feature = np.ascontiguousarray(np.asarray(feature, dtype=np.float32))
    embedding = np.ascontiguousarray(np.asarray(embedding, dtype=np.float32))
    weight = np.ascontiguousarray(np.asarray(weight, dtype=np.float32))

    f_pad = np.pad(feature, ((0, 0), (0, 0), (1, 1)))  # [B, C, T+2]
    parts = [f_pad[:, c0 : c0 + 128, :] for c0 in BIGS]
    for j in range(3):  # boundary chunks: rows are k-shifted channel copies
        rows = np.stack([f_pad[:, c, k : k + T] for (c, k) in D_PAIRS[j]], axis=1)
        parts.append(np.pad(rows, ((0, 0), (0, 0), (0, 2))))
    fmega = np.concatenate(
        [parts[s][:, :, a : a + w] for (a, w) in F_BLOCKS for s in range(6)],
        axis=2,
    ).astype(ml_dtypes.bfloat16)  # [B, 128, 24636]

    e_pad = np.pad(embedding, ((0, 0), (0, 0), (1, 0)))  # [B, C, T+1]
    e_packed = np.concatenate(
        [
            e_pad[:, 128 * p : 128 * p + 128, 1024 * kb : 1024 * kb + 1025]
            for kb in range(NKB)
            for p in range(CP)
        ],
        axis=2,
    ).astype(ml_dtypes.bfloat16)  # [B, 128, 16400]

    wch = []
    for j in range(3):
        c0 = BIGS[j]
        for k in range(K):
            wch.append(weight[:, c0 : c0 + 128, k].T)  # [128, O]
        wch.append(np.stack([weight[:, c, k] for (c, k) in D_PAIRS[j]], axis=0))
    # column order j0, j2, j1: the deferred-epilogue conv needs j0/j2 first,
    # j1 only after the sims land, so its DMA can trail the E blocks
    worder = [0, 1, 2, 3, 8, 9, 10, 11, 4, 5, 6, 7]
    wmega = np.concatenate([wch[ci] for ci in worder], axis=1).astype(
        ml_dtypes.bfloat16
    )  # [128, 6144]

    in_maps = [
        {
            "feature": np.ascontiguousarray(fmega[b]),
            "embedding": np.ascontiguousarray(e_packed[b]),
            "weight_t": wmega,
        }
        for b in range(B)
    ]
    return in_maps


def _fix_sync_waits(nc, limit=1):
    """Split instructions with more sem waits than walrus' TPB encoding allows."""
    counter = 0
    for f in nc.m.functions:
        for bb in f.blocks:
            insts = list(bb.instructions)
            new_insts = []
            changed = False
            for inst in insts:
                si = inst.sync_info
                waits = list(si.on_wait) if si and si.on_wait else []
                if len(waits) > limit:
                    changed = True
                    head, rest = waits[:-limit], waits[-limit:]
                    for i in range(0, len(head), limit):
                        counter += 1
                        nop = mybir.InstNoOp(name=f"I-waitsplit-{counter}")
                        nop.engine = inst.engine
                        nop.sync_info = mybir.SyncInfo(
                            on_wait=head[i : i + limit], on_update=[]
                        )
                        new_insts.append(nop)
                    inst.sync_info = mybir.SyncInfo(
                        on_wait=rest, on_update=list(si.on_update or [])
                    )
                new_insts.append(inst)
            if changed:
                bb.instructions.clear()
                for i in new_insts:
                    bb.add_instruction(i)
    return counter


NDEF = 12  # leading out-tiles run before the sims exist (deferred epilogue)


def build_kernel():
    nc = bass.Bass(target_bir_lowering=False, trn_type="TRN2")
    Fd = nc.declare_dram_parameter("feature", [128, F_COLS], BF16, isOutput=False)
    Ed = nc.declare_dram_parameter("embedding", [128, 16400], BF16, isOutput=False)
    Wd = nc.declare_dram_parameter("weight_t", [128, 6144], BF16, isOutput=False)
    Od = nc.declare_dram_parameter("out", [T, O], F32, isOutput=True)

    with tile.TileContext(nc) as tc, ExitStack() as ctx:
        body(ctx, tc, Fd, Ed, Wd, Od)
    _fix_sync_waits(nc, limit=1)
    return nc


def body(ctx, tc, Fd, Ed, Wd, Od):
    nc = tc.nc

    consts = ctx.enter_context(tc.tile_pool(name="consts", bufs=1))
    epool = ctx.enter_context(tc.tile_pool(name="epool", bufs=1))
    fpool = ctx.enter_context(tc.tile_pool(name="fpool", bufs=1))
    wpool = ctx.enter_context(tc.tile_pool(name="wpool", bufs=1))
    sqpool = ctx.enter_context(tc.tile_pool(name="sqpool", bufs=4))
    rowpool = ctx.enter_context(tc.tile_pool(name="rowpool", bufs=1))
    simpool = ctx.enter_context(tc.tile_pool(name="simpool", bufs=1))
    defpool = ctx.enter_context(tc.tile_pool(name="defpool", bufs=1))
    outpool = ctx.enter_context(tc.tile_pool(name="outpool", bufs=4))

    # --- constants ---
    ones_t = consts.tile([128, 128], BF16, tag="ones")
    nc.vector.memset(ones_t[:], 1.0)
    e0 = consts.tile([128, 1], BF16, tag="e0")
    nc.vector.memset(e0[:], 0.0)
    nc.vector.memset(e0[0:1, :], 1.0)

    # --- DMA priority order: the conv prefix (j0/j2 weights + first F blocks)
    # first so the PE starts ~5us after boot, then E for the sims, then the
    # j1 weights and remaining F.
    wt = wpool.tile([128, 6144], BF16, tag="wmega")
    fmega = fpool.tile([128, F_COLS], BF16, tag="fmega")
    nc.sync.dma_start(wt[:, 0:2048], Wd[:, 0:2048])  # j0 chunks
    a, b = _F_STARTS[0], _F_STARTS[1]
    nc.sync.dma_start(fmega[:, a:b], Fd[:, a:b])
    nc.sync.dma_start(wt[:, 2048:4096], Wd[:, 2048:4096])  # j2 chunks
    a, b = _F_STARTS[1], _F_STARTS[2]
    nc.sync.dma_start(fmega[:, a:b], Fd[:, a:b])
    e_kbs = []
    for kb in range(NKB):
        ekb = epool.tile([128, 4100], BF16, tag=f"ekb{kb}")
        nc.sync.dma_start(ekb[:], Ed[:, 4100 * kb : 4100 * kb + 4100])
        e_kbs.append(ekb)
    nc.sync.dma_start(wt[:, 4096:6144], Wd[:, 4096:6144])  # j1 chunks
    for fb in (2, 3, 4):
        a, b = _F_STARTS[fb], _F_STARTS[fb + 1]
        nc.sync.dma_start(fmega[:, a:b], Fd[:, a:b])

    # --- sim row space: sq/pl (bf16) -> ones-matmul partition reduce -> n/dl rows
    n_sb = rowpool.tile([128, T + 2], BF16, tag="n_sb")
    dl_sb = rowpool.tile([128, T + 2], BF16, tag="dl_sb")
    for sb in (n_sb, dl_sb):
        nc.vector.memset(sb[:, 0:1], 0.0)
        nc.vector.memset(sb[:, T + 1 : T + 2], 0.0)

    # --- conv helpers ---
    def w_ap(ci):
        c = W_COL[ci]
        return wt[0:128, c : c + 512]

    def f_ap(s, off, q):  # chunk-tile s at t-offset off for out tile q
        blk = next(i for i, (lo, hi) in enumerate(F_BLK_Q) if lo <= q < hi)
        base = _F_STARTS[blk] + F_BLOCKS[blk][1] * s + off + 128 * q - F_BLOCKS[blk][0]
        return fmega[0:128, base : base + 128]

    per_j = [
        [(j, k, 4 * j + k) for k in range(K)] + [(3 + j, 0, 4 * j + 3)]
        for j in range(3)
    ]

    def mm_group(psum, j, q):
        cl = per_j[j]
        for idx, (s, off, ci) in enumerate(cl):
            nc.tensor.matmul(
                psum[:],
                f_ap(s, off, q),
                w_ap(ci),
                start=(idx == 0),
                stop=(idx == len(cl) - 1),
            )

    def epilogue(q, p0, p1, p2):
        # ACT: tmp = P0 * sim_l; DVE: tmp2 = (P2 * sim_r) + tmp; osb = P1 + tmp2
        # sim_l[q] lives in xt_sb col q, sim_r[q] in col TQ+q (per-partition).
        tmp = outpool.tile([128, O], F32, tag="tmp", name=f"tmp{q}")
        nc.scalar.mul(tmp[:], p0, xt_sb[:, q : q + 1])
        tmp2 = outpool.tile([128, O], F32, tag="tmp2", name=f"tmp2_{q}")
        nc.vector.scalar_tensor_tensor(
            tmp2[:], p2, xt_sb[:, TQ + q : TQ + q + 1], tmp[:],
            op0=mybir.AluOpType.mult, op1=mybir.AluOpType.add,
        )
        osb = outpool.tile([128, O], F32, tag="osb", name=f"osb{q}")
        nc.vector.tensor_add(osb[:], p1, tmp2[:])
        nc.sync.dma_start(Od[128 * q : 128 * q + 128, :], osb[:])

    def red_kb(kb, s2b, dlb):
        for p in range(CP):
            esl = e_kbs[kb][:, 1025 * p : 1025 * p + 1025]
            sq = sqpool.tile([128, 1024], BF16, tag="sq", name=f"sq{kb}_{p}")
            pl = sqpool.tile([128, 1024], BF16, tag="pl", name=f"pl{kb}_{p}")
            if p < 2:  # split squares across ACT and DVE
                nc.scalar.square(sq[:], esl[:, 1:1025])
            else:
                nc.vector.tensor_mul(sq[:], esl[:, 1:1025], esl[:, 1:1025])
            nc.vector.tensor_mul(pl[:], esl[:, 1:1025], esl[:, 0:1024])
            for h in range(2):  # matmul out limited to one PSUM bank (512 f32)
                hs = slice(512 * h, 512 * h + 512)
                nc.tensor.matmul(
                    s2b[:, hs], ones_t[:], sq[:, hs],
                    start=(p == 0), stop=(p == CP - 1),
                )
                nc.tensor.matmul(
                    dlb[:, hs], ones_t[:], pl[:, hs],
                    start=(p == 0), stop=(p == CP - 1),
                )
        # evacuate: n = sqrt(s2) via ACT, dl plain copy via DVE (bf16 out)
        nc.scalar.sqrt(n_sb[:, 1 + 1024 * kb : 1025 + 1024 * kb], s2b[:])
        nc.vector.tensor_copy(dl_sb[:, 1 + 1024 * kb : 1025 + 1024 * kb], dlb[:])

    # --- phase 1: deferred conv tiles (j0/j2 now, bf16 SBUF stash) with the
    # sim reduce interleaved into the PE stream as E kilo-blocks arrive.
    a_sbs, c_sbs = {}, {}
    with tc.tile_pool(name="dpsum", bufs=2, space="PSUM") as dpsum, tc.tile_pool(
        name="redpsum", bufs=1, space="PSUM"
    ) as redpsum:
        # HAM warm-up: dummy 1-col matmuls while the first W/F DMAs stream, so
        # the PE clock gate is already at 8/8 when the real conv begins.
        wps = dpsum.tile([128, O], F32, tag="PA", name="warmps")
        for i in range(35):
            nc.tensor.matmul(wps[:, 0:1], ones_t[:], e0[:], start=True, stop=True)
        red_after = {3: 0, 4: 1, 5: 2, 6: 3}  # deferred-tile idx -> kb
        for q in range(NDEF):
            pa = dpsum.tile([128, O], F32, tag="PA", name=f"PA{q}")
            mm_group(pa, 0, q)
            a_sb = defpool.tile([128, O], BF16, tag=f"dA{q}", name=f"dA{q}")
            nc.scalar.copy(a_sb[:], pa[:])
            a_sbs[q] = a_sb
            pc = dpsum.tile([128, O], F32, tag="PC", name=f"PC{q}")
            mm_group(pc, 2, q)
            c_sb = defpool.tile([128, O], BF16, tag=f"dC{q}", name=f"dC{q}")
            nc.vector.tensor_copy(c_sb[:], pc[:])
            c_sbs[q] = c_sb
            if q in red_after:
                kb = red_after[q]
                s2b = redpsum.tile([128, 1024], F32, tag="s2b", name=f"s2b{kb}")
                dlb = redpsum.tile([128, 1024], F32, tag="dlb", name=f"dlb{kb}")
                red_kb(kb, s2b, dlb)
        # sims in ROW space (DVE, bf16, in-place buffer per sim), then a single
        # transposing extraction per sim row:
        #   sim_l[t] = dl[t] / max(n[t]*n[t-1], tiny)
        #   sim_r[t] = dl[t+1] / max(n[t]*n[t+1], tiny)
        simrow_specs = (
            ("sl", 0, 1),  # (tag, n-shift for the product, dl col offset)
            ("sr", 2, 2),
        )
        sim_rows = []
        with nc.allow_low_precision(reason="sims scale small conv terms; bf16 ok"):
            for tag, nsh, doff in simrow_specs:
                srow = rowpool.tile(
                    [128, T], BF16, tag=f"srow_{tag}", name=f"srow_{tag}"
                )
                nc.vector.tensor_mul(
                    srow[:], n_sb[:, 1 : T + 1], n_sb[:, nsh : nsh + T]
                )
                nc.vector.tensor_scalar_max(srow[:], srow[:], 1e-30)
                nc.vector.reciprocal(srow[:], srow[:])
                nc.vector.tensor_mul(srow[:], dl_sb[:, doff : doff + T], srow[:])
                sim_rows.append(srow)
        # extract sim rows into transposed [t mod 128, q] layout
        xt_all = dpsum.tile([128, 2 * TQ], F32, tag="PA", name="xt_all")
        for v, srow in enumerate(sim_rows):
            for q in range(TQ):
                nc.tensor.matmul(
                    xt_all[:, 32 * v + q : 32 * v + q + 1],
                    srow[:, 128 * q : 128 * q + 128],
                    e0[:],
                    start=True,
                    stop=True,
                )
        xt_sb = simpool.tile([128, 2 * TQ], F32, tag="xt_sb", name="xt_sb")
        nc.vector.tensor_copy(xt_sb[:], xt_all[:])

    # --- phase 2: finish deferred tiles (j1 matmuls + epilogue) interleaved
    # with the remaining live tiles.
    convpsum = ctx.enter_context(tc.tile_pool(name="convpsum", bufs=2, space="PSUM"))

    def p1_finish(q):
        p1 = convpsum.tile([128, O], F32, tag="P1", name=f"P1d{q}")
        mm_group(p1, 1, q)
        epilogue(q, a_sbs[q][:], p1[:], c_sbs[q][:])

    def live_tile(q):
        psums = [
            convpsum.tile([128, O], F32, tag=f"P{j}", name=f"P{j}_{q}")
            for j in range(3)
        ]
        for j in range(3):
            mm_group(psums[j], j, q)
        epilogue(q, psums[0][:], psums[1][:], psums[2][:])

    todo = list(range(NDEF))
    for q in range(NDEF, TQ):
        if todo:
            p1_finish(todo.pop(0))
        live_tile(q)
    for q in todo:
        p1_finish(q)


_NC_CACHE = {}


def _get_nc():
    if "nc" not in _NC_CACHE:
        _NC_CACHE["nc"] = build_kernel()
    return _NC_CACHE["nc"]


def kernel(feature, embedding, weight):
    in_maps = host_prep(feature, embedding, weight)
    nc = _get_nc()
    res = run_bass_kernel_spmd(nc, in_maps, core_ids=list(range(B)))
    out = np.stack([res.results[b]["out"].T for b in range(B)])  # [B, O, T]
    return np.ascontiguousarray(out)


# revision 24
# speedup vs baseline: 1.4473x; 1.4473x over previous
"""AttentionConv1d Trainium2 kernel — 8-core batch-parallel SPMD.

Reference semantics (B=8, C=512, T=4096, O=512, K=3):
    out[b,o,t] = sum_{c,k} feature[b,c,t+k-1] * sim[b, (3c+k)//512, t] * weight[o,c,k]
where sim[b,0/1/2,t] are cosine similarities of embedding column t with its
left neighbor / itself / right neighbor (zero-padded at the edges), so
sim[:,1,:] == 1 for any column with norm >= eps.

Decomposition per batch element (one NeuronCore each):
    conv_j[o,t] = sum over the (c,k) pairs with (3c+k)//512 == j
    out = sim_l * conv_0 + conv_1 + sim_r * conv_2
computed transposed (out_T[t,o]) so sim_l/sim_r become per-partition scalars.

Exact 12-chunk packing: each j-group's 512 (c,k) pairs split into 3 dense
128-channel chunks (k = 0,1,2 column offsets of one resident F tile) plus one
128-row boundary chunk whose rows are host-side k-shifted copies — no masked
zero rows, 12 matmuls of 128 contraction rows per 128-t out tile (the 1536-row
minimum). DMA priority: E (bf16) first so the sim reduce starts immediately,
then W, then F in column blocks so conv tile q never waits. The conv runs in
bf16 (host-cast), PSUM accumulation in fp32.
"""
from contextlib import ExitStack

import ml_dtypes
import numpy as np

import concourse.bass as bass
import concourse.tile as tile
from concourse import mybir
from concourse.bass_utils import run_bass_kernel_spmd

F32 = mybir.dt.float32
BF16 = mybir.dt.bfloat16

B, C, T, O, K = 8, 512, 4096, 512, 3
CP = C // 128  # 4 c-tiles
TQ = T // 128  # 32 t-tiles
NKB = T // 1024  # 4 reduce kilo-blocks

# j-group (3c+k)//512 channel structure: dense 128-channel blocks + 128
# boundary pairs per group (exactly 512 pairs per group, 1536 total).
BIGS = [0, 171, 384]
D_PAIRS = [
    [(c, k) for c in range(128, 170) for k in range(3)] + [(170, 0), (170, 1)],
    [(170, 2)] + [(c, k) for c in range(299, 341) for k in range(3)] + [(341, 0)],
    [(341, 1), (341, 2)] + [(c, k) for c in range(342, 384) for k in range(3)],
]
assert all(len(p) == 128 for p in D_PAIRS)

# F column blocks (start, width) in f_pad coords; blocks overlap 2 cols so any
# 130-col conv window lies inside one block. First block is small so the conv
# can start as soon as ~0.4MB of F has landed.
F_BLOCKS = [(0, 258), (256, 770), (1024, 1026), (2048, 1026), (3072, 1026)]
F_BLK_Q = [(0, 2), (2, 8), (8, 16), (16, 24), (24, 32)]  # q-tile range per block
_F_STARTS = [0]
for _a, _w in F_BLOCKS:
    _F_STARTS.append(_F_STARTS[-1] + 6 * _w)  # fmega col start per block
F_COLS = _F_STARTS[-1]  # 24636

# wmega chunk order: j0 (4), j2 (4), j1 (4)
W_COL = {}
for _i, _ci in enumerate([0, 1, 2, 3, 8, 9, 10, 11, 4, 5, 6, 7]):
    W_COL[_ci] = 512 * _i


def host_prep(feature, embedding, weight):
    """Build per-core input maps: packed F/E shards + exact packed weights.

    Everything is packed into 128-partition-row mega layouts so each tensor
    needs only a handful of dma_start instructions (each costs ~0.6us of sync
    engine issue time on HW):
      e_packed [128, 16400]: 4 kilo-blocks x 4 c-tiles x 1025 cols (bf16)
      fmega    [128, 24624]: 4 col-blocks x 6 chunk-tiles x 1026 cols, blocks
               overlap 2 cols so any 130-col conv window stays inside a block
      wmega    [128, 6144]: 12 chunks x 512 out-channels
    """
    feature = np.ascontiguousarray(np.asarray(feature, dtype=np.float32))
    embedding = np.ascontiguousarray(np.asarray(embedding, dtype=np.float32))
    weight = np.ascontiguousarray(np.asarray(weight, dtype=np.float32))

    f_pad = np.pad(feature, ((0, 0), (0, 0), (1, 1)))  # [B, C, T+2]
    parts = [f_pad[:, c0 : c0 + 128, :] for c0 in BIGS]
    for j in range(3):  # boundary chunks: rows are k-shifted channel copies
        rows = np.stack([f_pad[:, c, k : k + T] for (c, k) in D_PAIRS[j]], axis=1)
        parts.append(np.pad(rows, ((0, 0), (0, 0), (0, 2))))
    fmega = np.concatenate(
        [parts[s][:, :, a : a + w] for (a, w) in F_BLOCKS for s in range(6)],
        axis=2,
    ).astype(ml_dtypes.bfloat16)  # [B, 128, 24636]

    e_pad = np.pad(embedding, ((0, 0), (0, 0), (1, 0)))  # [B, C, T+1]
    e_packed = np.concatenate(
        [
            e_pad[:, 128 * p : 128 * p + 128, 1024 * kb : 1024 * kb + 1025]
            for kb in range(NKB)
            for p in range(CP)
        ],
        axis=2,
    ).astype(ml_dtypes.bfloat16)  # [B, 128, 16400]

    wch = []
    for j in range(3):
        c0 = BIGS[j]
        for k in range(K):
            wch.append(weight[:, c0 : c0 + 128, k].T)  # [128, O]
        wch.append(np.stack([weight[:, c, k] for (c, k) in D_PAIRS[j]], axis=0))
    # column order j0, j2, j1: the deferred-epilogue conv needs j0/j2 first,
    # j1 only after the sims land, so its DMA can trail the E blocks
    worder = [0, 1, 2, 3, 8, 9, 10, 11, 4, 5, 6, 7]
    wmega = np.concatenate([wch[ci] for ci in worder], axis=1).astype(
        ml_dtypes.bfloat16
    )  # [128, 6144]

    in_maps = [
        {
            "feature": np.ascontiguousarray(fmega[b]),
            "embedding": np.ascontiguousarray(e_packed[b]),
            "weight_t": wmega,
        }
        for b in range(B)
    ]
    return in_maps


def _fix_sync_waits(nc, limit=1):
    """Split instructions with more sem waits than walrus' TPB encoding allows."""
    counter = 0
    for f in nc.m.functions:
        for bb in f.blocks:
            insts = list(bb.instructions)
            new_insts = []
            changed = False
            for inst in insts:
                si = inst.sync_info
                waits = list(si.on_wait) if si and si.on_wait else []
                if len(waits) > limit:
                    changed = True
                    head, rest = waits[:-limit], waits[-limit:]
                    for i in range(0, len(head), limit):
                        counter += 1
                        nop = mybir.InstNoOp(name=f"I-waitsplit-{counter}")
                        nop.engine = inst.engine
                        nop.sync_info = mybir.SyncInfo(
                            on_wait=head[i : i + limit], on_update=[]
                        )
                        new_insts.append(nop)
                    inst.sync_info = mybir.SyncInfo(
                        on_wait=rest, on_update=list(si.on_update or [])
                    )
                new_insts.append(inst)
            if changed:
                bb.instructions.clear()
                for i in new_insts:
                    bb.add_instruction(i)
    return counter


NDEF = 12  # leading out-tiles run before the sims exist (deferred epilogue)


def build_kernel():
    nc = bass.Bass(target_bir_lowering=False, trn_type="TRN2")
    Fd = nc.declare_dram_parameter("feature", [128, F_COLS], BF16, isOutput=False)
    Ed = nc.declare_dram_parameter("embedding", [128, 16400], BF16, isOutput=False)
    Wd = nc.declare_dram_parameter("weight_t", [128, 6144], BF16, isOutput=False)
    Od = nc.declare_dram_parameter("out", [T, O], F32, isOutput=True)

    with tile.TileContext(nc) as tc, ExitStack() as ctx:
        body(ctx, tc, Fd, Ed, Wd, Od)
    _fix_sync_waits(nc, limit=1)
    return nc


def body(ctx, tc, Fd, Ed, Wd, Od):
    nc = tc.nc

    consts = ctx.enter_context(tc.tile_pool(name="consts", bufs=1))
    epool = ctx.enter_context(tc.tile_pool(name="epool", bufs=1))
    fpool = ctx.enter_context(tc.tile_pool(name="fpool", bufs=1))
    wpool = ctx.enter_context(tc.tile_pool(name="wpool", bufs=1))
    sqpool = ctx.enter_context(tc.tile_pool(name="sqpool", bufs=4))
    rowpool = ctx.enter_context(tc.tile_pool(name="rowpool", bufs=1))
    simpool = ctx.enter_context(tc.tile_pool(name="simpool", bufs=1))
    defpool = ctx.enter_context(tc.tile_pool(name="defpool", bufs=1))
    outpool = ctx.enter_context(tc.tile_pool(name="outpool", bufs=4))

    # --- constants ---
    ones_t = consts.tile([128, 128], BF16, tag="ones")
    nc.vector.memset(ones_t[:], 1.0)
    e0 = consts.tile([128, 1], BF16, tag="e0")
    nc.vector.memset(e0[:], 0.0)
    nc.vector.memset(e0[0:1, :], 1.0)

    # --- DMA priority order: the conv prefix (j0/j2 weights + first F blocks)
    # first so the PE starts ~5us after boot, then E for the sims, then the
    # j1 weights and remaining F.
    wt = wpool.tile([128, 6144], BF16, tag="wmega")
    fmega = fpool.tile([128, F_COLS], BF16, tag="fmega")
    nc.sync.dma_start(wt[:, 0:2048], Wd[:, 0:2048])  # j0 chunks
    a, b = _F_STARTS[0], _F_STARTS[1]
    nc.sync.dma_start(fmega[:, a:b], Fd[:, a:b])
    nc.sync.dma_start(wt[:, 2048:4096], Wd[:, 2048:4096])  # j2 chunks
    a, b = _F_STARTS[1], _F_STARTS[2]
    nc.sync.dma_start(fmega[:, a:b], Fd[:, a:b])
    e_kbs = []
    for kb in range(NKB):
        ekb = epool.tile([128, 4100], BF16, tag=f"ekb{kb}")
        nc.sync.dma_start(ekb[:], Ed[:, 4100 * kb : 4100 * kb + 4100])
        e_kbs.append(ekb)
    nc.sync.dma_start(wt[:, 4096:6144], Wd[:, 4096:6144])  # j1 chunks
    for fb in (2, 3, 4):
        a, b = _F_STARTS[fb], _F_STARTS[fb + 1]
        nc.sync.dma_start(fmega[:, a:b], Fd[:, a:b])

    # --- sim row space: sq/pl (bf16) -> ones-matmul partition reduce -> n/dl rows
    n_sb = rowpool.tile([128, T + 2], BF16, tag="n_sb")
    dl_sb = rowpool.tile([128, T + 2], BF16, tag="dl_sb")
    for sb in (n_sb, dl_sb):
        nc.vector.memset(sb[:, 0:1], 0.0)
        nc.vector.memset(sb[:, T + 1 : T + 2], 0.0)

    # --- conv helpers ---
    def w_ap(ci):
        c = W_COL[ci]
        return wt[0:128, c : c + 512]

    def f_ap(s, off, q):  # chunk-tile s at t-offset off for out tile q
        blk = next(i for i, (lo, hi) in enumerate(F_BLK_Q) if lo <= q < hi)
        base = _F_STARTS[blk] + F_BLOCKS[blk][1] * s + off + 128 * q - F_BLOCKS[blk][0]
        return fmega[0:128, base : base + 128]

    per_j = [
        [(j, k, 4 * j + k) for k in range(K)] + [(3 + j, 0, 4 * j + 3)]
        for j in range(3)
    ]

    def mm_group(psum, j, q):
        cl = per_j[j]
        for idx, (s, off, ci) in enumerate(cl):
            nc.tensor.matmul(
                psum[:],
                f_ap(s, off, q),
                w_ap(ci),
                start=(idx == 0),
                stop=(idx == len(cl) - 1),
            )

    def epilogue(q, p0, p1, p2):
        # ACT: tmp = P0 * sim_l; DVE: tmp2 = (P2 * sim_r) + tmp; osb = P1 + tmp2
        # sim_l[q] lives in xt_sb col q, sim_r[q] in col TQ+q (per-partition).
        tmp = outpool.tile([128, O], F32, tag="tmp", name=f"tmp{q}")
        nc.scalar.mul(tmp[:], p0, xt_sb[:, q : q + 1])
        tmp2 = outpool.tile([128, O], F32, tag="tmp2", name=f"tmp2_{q}")
        nc.vector.scalar_tensor_tensor(
            tmp2[:], p2, xt_sb[:, TQ + q : TQ + q + 1], tmp[:],
            op0=mybir.AluOpType.mult, op1=mybir.AluOpType.add,
        )
        osb = outpool.tile([128, O], F32, tag="osb", name=f"osb{q}")
        nc.vector.tensor_add(osb[:], p1, tmp2[:])
        nc.sync.dma_start(Od[128 * q : 128 * q + 128, :], osb[:])

    def red_kb(kb, s2b, dlb):
        for p in range(CP):
            esl = e_kbs[kb][:, 1025 * p : 1025 * p + 1025]
            sq = sqpool.tile([128, 1024], BF16, tag="sq", name=f"sq{kb}_{p}")
            pl = sqpool.tile([128, 1024], BF16, tag="pl", name=f"pl{kb}_{p}")
            if p < 2:  # split squares across ACT and DVE
                nc.scalar.square(sq[:], esl[:, 1:1025])
            else:
                nc.vector.tensor_mul(sq[:], esl[:, 1:1025], esl[:, 1:1025])
            nc.vector.tensor_mul(pl[:], esl[:, 1:1025], esl[:, 0:1024])
            for h in range(2):  # matmul out limited to one PSUM bank (512 f32)
                hs = slice(512 * h, 512 * h + 512)
                nc.tensor.matmul(
                    s2b[:, hs], ones_t[:], sq[:, hs],
                    start=(p == 0), stop=(p == CP - 1),
                )
                nc.tensor.matmul(
                    dlb[:, hs], ones_t[:], pl[:, hs],
                    start=(p == 0), stop=(p == CP - 1),
                )
        # evacuate: n = sqrt(s2) via ACT, dl plain copy via DVE (bf16 out)
        nc.scalar.sqrt(n_sb[:, 1 + 1024 * kb : 1025 + 1024 * kb], s2b[:])
        nc.vector.tensor_copy(dl_sb[:, 1 + 1024 * kb : 1025 + 1024 * kb], dlb[:])

    # --- phase 1: deferred conv tiles (j0/j2 now, bf16 SBUF stash) with the
    # sim reduce interleaved into the PE stream as E kilo-blocks arrive.
    a_sbs, c_sbs = {}, {}
    with tc.tile_pool(name="dpsum", bufs=2, space="PSUM") as dpsum, tc.tile_pool(
        name="redpsum", bufs=1, space="PSUM"
    ) as redpsum:
        # HAM warm-up: dummy 1-col matmuls while the first W/F DMAs stream, so
        # the PE clock gate is already at 8/8 when the real conv begins.
        wps = dpsum.tile([128, O], F32, tag="PA", name="warmps")
        for i in range(35):
            nc.tensor.matmul(wps[:, 0:1], ones_t[:], e0[:], start=True, stop=True)
        red_after = {3: 0, 4: 1, 5: 2, 6: 3}  # deferred-tile idx -> kb
        for q in range(NDEF):
            pa = dpsum.tile([128, O], F32, tag="PA", name=f"PA{q}")
            mm_group(pa, 0, q)
            a_sb = defpool.tile([128, O], BF16, tag=f"dA{q}", name=f"dA{q}")
            nc.scalar.copy(a_sb[:], pa[:])
            a_sbs[q] = a_sb
            pc = dpsum.tile([128, O], F32, tag="PC", name=f"PC{q}")
            mm_group(pc, 2, q)
            c_sb = defpool.tile([128, O], BF16, tag=f"dC{q}", name=f"dC{q}")
            nc.vector.tensor_copy(c_sb[:], pc[:])
            c_sbs[q] = c_sb
            if q in red_after:
                kb = red_after[q]
                s2b = redpsum.tile([128, 1024], F32, tag="s2b", name=f"s2b{kb}")
                dlb = redpsum.tile([128, 1024], F32, tag="dlb", name=f"dlb{kb}")
                red_kb(kb, s2b, dlb)
        # window-extraction: n_sb col 1+t = n[t]; dl_sb col 1+t = dl[t]
        xt_all = dpsum.tile([128, 5 * TQ], F32, tag="PA", name="xt_all")
        variants = [
            ("nT0", n_sb, 1),
            ("nTm", n_sb, 0),
            ("nTp", n_sb, 2),
            ("dT0", dl_sb, 1),
            ("dTp", dl_sb, 2),
        ]
        for v, (name, src, off) in enumerate(variants):
            for q in range(TQ):
                nc.tensor.matmul(
                    xt_all[:, 32 * v + q : 32 * v + q + 1],
                    src[:, off + 128 * q : off + 128 * q + 128],
                    e0[:],
                    start=True,
                    stop=True,
                )
        xt_raw = simpool.tile([128, 5 * TQ], F32, tag="xt_raw", name="xt_raw")
        nc.vector.tensor_copy(xt_raw[:], xt_all[:])
    cols = {
        name: xt_raw[:, 32 * v : 32 * v + 32]
        for v, (name, _, _) in enumerate(variants)
    }

    # sims on tiny transposed tiles; pack [sim_l | sim_r] into xt_sb so the
    # epilogue can address sim_l[q] = col q, sim_r[q] = col TQ+q.
    xt_sb = simpool.tile([128, 2 * TQ], F32, tag="xt_sb", name="xt_sb")
    for i, (nx, dx) in enumerate((("nTm", "dT0"), ("nTp", "dTp"))):
        sl = xt_sb[:, TQ * i : TQ * i + TQ]
        prod = simpool.tile([128, TQ], F32, tag=f"prod_{nx}", name=f"prod_{nx}")
        nc.vector.tensor_mul(prod[:], cols["nT0"], cols[nx])
        nc.vector.tensor_scalar_max(prod[:], prod[:], 1e-30)
        nc.vector.reciprocal(prod[:], prod[:])
        nc.vector.tensor_mul(sl, cols[dx], prod[:])

    # --- phase 2: finish deferred tiles (j1 matmuls + epilogue) interleaved
    # with the remaining live tiles.
    convpsum = ctx.enter_context(tc.tile_pool(name="convpsum", bufs=2, space="PSUM"))

    def p1_finish(q):
        p1 = convpsum.tile([128, O], F32, tag="P1", name=f"P1d{q}")
        mm_group(p1, 1, q)
        epilogue(q, a_sbs[q][:], p1[:], c_sbs[q][:])

    def live_tile(q):
        psums = [
            convpsum.tile([128, O], F32, tag=f"P{j}", name=f"P{j}_{q}")
            for j in range(3)
        ]
        for j in range(3):
            mm_group(psums[j], j, q)
        epilogue(q, psums[0][:], psums[1][:], psums[2][:])

    todo = list(range(NDEF))
    for q in range(NDEF, TQ):
        if todo:
            p1_finish(todo.pop(0))
        live_tile(q)
    for q in todo:
        p1_finish(q)


_NC_CACHE = {}


def _get_nc():
    if "nc" not in _NC_CACHE:
        _NC_CACHE["nc"] = build_kernel()
    return _NC_CACHE["nc"]


def kernel(feature, embedding, weight):
    in_maps = host_prep(feature, embedding, weight)
    nc = _get_nc()
    res = run_bass_kernel_spmd(nc, in_maps, core_ids=list(range(B)))
    out = np.stack([res.results[b]["out"].T for b in range(B)])  # [B, O, T]
    return np.ascontiguousarray(out)
